# revision 14
# baseline (speedup 1.0000x reference)
"""Distributed 1D attention kernel for Trainium2 (8 NeuronCores).

Problem: x [4,256,2048], y [4,256,2048] ->
  q = Wq@x, k = Wk@y, v = Wv@y  (per-head d=128, H=8 heads)
  out = Wo @ concat_h(softmax(q^T k / sqrt(128)) applied to v)   -> [4,128,2048]

Sharding: core = 2*b + g where b in [0,4) is the batch and g in {0,1} picks
heads [4g, 4g+4). Each core computes its 4 (b,h) attention pairs plus the
partial Wo projection for its head group; the host sums the two partials
per batch.

Device-side layout (per core), fp16 operands / f32 PSUM:
  logitsT tile [y=128p, x=512] = matmul(lhsT=k_h[d, ytile], rhs=q_h[d, xblk])
  exp on ScalarE (PSUM->SBUF fp16, scale=1/sqrt(128) folded in; logits are
  ~N(0,1) so no max subtraction is needed) into one E mega-tile [128,16,512]
  AV:    out_raw[d, x] = sum_yt matmul(lhsT=vT[ytile, d_h], rhs=E[:, yt, :])
  denom: DVE binary add-tree over the 16 E y-tiles -> TS[128,512], then ONE
         matmul(lhsT=ones[128,128], rhs=TS) broadcasts sum_y across
         partitions (the 8 ones-matmuls + pair-sums of the bf16 version
         cost ~24us more PE time; the PE is this kernel's critical engine)
  recip: vector.reciprocal_approx_fast (18 bits; den ~3400 so edge-safe)
  normalize on DVE: att[d, x] = out_raw * (1/den)
  Wo:    out[o, x] = sum_h matmul(lhsT=WoT[hd tile, o], rhs=att[hd, x])
K/V/AV-output casts run on GpSimd (idle engine) to keep DVE off the
critical path during the projection prologue.
"""

import sys

if "/opt/trn_rl_repo" not in sys.path:
    sys.path.insert(0, "/opt/trn_rl_repo")

import numpy as np


def _install_ntff_shim():
    """antenv.axon_hooks is absent from this image, which crashes
    run_bass_kernel_spmd(trace=True). Recreate it from the hook factory
    that trn_agent_boot ships."""
    import types

    if "antenv.axon_hooks" in sys.modules:
        return
    mod = types.ModuleType("antenv.axon_hooks")
    _hook = [None]
    mod.set_axon_ntff_profile_hook = lambda h: _hook.__setitem__(0, h)
    mod.get_axon_ntff_profile_hook = lambda: _hook[0]
    sys.modules["antenv.axon_hooks"] = mod
    try:
        import antenv

        antenv.axon_hooks = mod
    except ImportError:
        pass
    try:
        from trn_agent_boot.trn_boot import _ntff_profile_via_ctypes

        mod.set_axon_ntff_profile_hook(
            _ntff_profile_via_ctypes("/opt/axon/libaxon_pjrt.so")
        )
    except Exception:
        pass


_install_ntff_shim()

import concourse.bass as bass
import concourse.mybir as mybir
import concourse.tile as tile
from concourse.bass_utils import run_bass_kernel_spmd

B, C, N, H, D = 4, 256, 2048, 8, 128
HPC = H // 2  # heads per core
NCORES = 8
F16 = mybir.dt.float16
F32 = mybir.dt.float32
NYT = N // 128  # 16 y tiles
NXB = N // 512  # 4 x blocks
SCALE = 1.0 / float(np.sqrt(D))

LAST_EXEC_NS = None
LAST_RESULTS = None


def _split_multi_waits(nc):
    """This walrus build accepts at most ONE sync wait per instruction;
    Tile's semaphore assignment attaches several. Hoist the extras into
    standalone event-semaphore instructions on the same engine."""
    ctr = 0
    for fn in nc.m.functions:
        for blk in fn.blocks:
            new_list = []
            changed = False
            for inst in blk.instructions:
                si = inst.sync_info
                if si is not None and len(si.on_wait) > 1:
                    waits = list(si.on_wait)
                    ups = list(si.on_update)
                    for w in waits[:-1]:
                        ev = mybir.InstEventSemaphore(
                            name=f"waitsplit-{ctr}", ins=[], outs=[]
                        )
                        ctr += 1
                        ev.engine = inst.engine
                        ev.sync_info = mybir.SyncInfo(on_wait=[w], on_update=[])
                        new_list.append(ev)
                    inst.sync_info = mybir.SyncInfo(on_wait=[waits[-1]], on_update=ups)
                    changed = True
                new_list.append(inst)
            if changed:
                blk.instructions = new_list
    return ctr


def _build_nc():
    nc = bass.Bass("TRN2", target_bir_lowering=False, debug=False)

    xb = nc.dram_tensor("xb", [C, N], F16, kind="ExternalInput")
    yb = nc.dram_tensor("yb", [C, N], F16, kind="ExternalInput")
    # wpack = [WKT | WQT | WVT] along the output dim, [c, 3*hd]
    wpack = nc.dram_tensor("wpack", [C, 3 * HPC * D], F16, kind="ExternalInput")
    wot = nc.dram_tensor("wot", [HPC * D, D], F16, kind="ExternalInput")  # [hd, o]
    out = nc.dram_tensor("out", [D, N], F32, kind="ExternalOutput")

    EXPF = mybir.ActivationFunctionType.Exp

    with tile.TileContext(nc) as tc:
        with (
            tc.tile_pool(name="w", bufs=1) as wpool,
            tc.tile_pool(name="big", bufs=1) as bigpool,
            tc.tile_pool(name="e", bufs=2) as epool,
            tc.tile_pool(name="t4", bufs=2) as t4pool,
            tc.tile_pool(name="t2", bufs=2) as t2pool,
            tc.tile_pool(name="t1", bufs=2) as t1pool,
            tc.tile_pool(name="ts", bufs=2) as tspool,
            tc.tile_pool(name="rc", bufs=2) as rcpool,
            tc.tile_pool(name="att", bufs=4) as attpool,
            tc.tile_pool(name="small", bufs=4) as smallpool,
            tc.tile_pool(name="pl", bufs=2, space="PSUM") as plpool,
            tc.tile_pool(name="po", bufs=2, space="PSUM") as popool,
            tc.tile_pool(name="pd", bufs=2, space="PSUM") as pdpool,
        ):
            # ---- input loads: one DMA per tensor, weights packed -----------
            xr = xb.rearrange("(kt p) n -> p kt n", p=128)
            yr = yb.rearrange("(kt p) n -> p kt n", p=128)
            wpr = wpack.rearrange("(kt p) m -> p kt m", p=128)
            Y = bigpool.tile([128, 2, N], F16, tag="Y")
            nc.sync.dma_start(Y[:], yr[:, :, :])
            WP = wpool.tile([128, 2, 3 * HPC * D], F16, tag="WP")
            nc.sync.dma_start(WP[:, :, 0 : HPC * D], wpr[:, :, 0 : HPC * D])
            nc.sync.dma_start(
                WP[:, :, HPC * D : 3 * HPC * D], wpr[:, :, HPC * D : 3 * HPC * D]
            )
            WKT = WP[:, :, 0 : HPC * D]
            WQT = WP[:, :, HPC * D : 2 * HPC * D]
            WVT = WP[:, :, 2 * HPC * D : 3 * HPC * D]
            X = bigpool.tile([128, 2, N], F16, tag="X")
            nc.sync.dma_start(X[:], xr[:, :, :])
            WOT = wpool.tile([128, HPC, D], F16, tag="WOT")
            nc.sync.dma_start(WOT[:], wot.rearrange("(h p) o -> p h o", p=128))
            ONES = wpool.tile([128, 128], F16, tag="ONES")
            nc.gpsimd.memset(ONES[:], 1.0)
            # HAM warm-up: keep the PE clock-gate open while input DMAs run,
            # so the first real matmuls start at 2.4 GHz instead of 1.2.
            WARM = plpool.tile([128, 1024], F32, tag="pl", name="warm")
            for _wi in range(60):
                nc.tensor.matmul(
                    WARM[:, :128], ONES[:], ONES[:], start=True, stop=True
                )

            # ---- projections ------------------------------------------------
            # h0's k/q and all of v run up front; proj_qk(1..3) interleaves
            # into the first slots (the slot order is h-major so head h isn't
            # needed until slot 4h). Casts are emitted 1024-wide on DVE.
            Q = bigpool.tile([128, HPC, N], F16, tag="Q")
            K = bigpool.tile([128, HPC, N], F16, tag="K")
            VT = bigpool.tile([128, NYT, HPC * D], F16, tag="VT")

            def proj_qk_half(h, half):
                hs = slice(h * 128, (h + 1) * 128)
                ns = slice(half * 1024, (half + 1) * 1024)
                pk = plpool.tile([128, 2, 512], F32, tag="pl", name=f"pk_{h}_{half}")
                pq = plpool.tile([128, 2, 512], F32, tag="pl", name=f"pq_{h}_{half}")
                for nb in range(2):
                    ns5 = slice(half * 1024 + nb * 512, half * 1024 + (nb + 1) * 512)
                    nc.tensor.matmul(
                        pk[:, nb, :], WKT[:, 0, hs], Y[:, 0, ns5], start=True, stop=False
                    )
                    nc.tensor.matmul(
                        pk[:, nb, :], WKT[:, 1, hs], Y[:, 1, ns5], start=False, stop=True
                    )
                for nb in range(2):
                    ns5 = slice(half * 1024 + nb * 512, half * 1024 + (nb + 1) * 512)
                    nc.tensor.matmul(
                        pq[:, nb, :], WQT[:, 0, hs], X[:, 0, ns5], start=True, stop=False
                    )
                    nc.tensor.matmul(
                        pq[:, nb, :], WQT[:, 1, hs], X[:, 1, ns5], start=False, stop=True
                    )
                nc.vector.tensor_copy(K[:, h, ns], pk[:])
                nc.vector.tensor_copy(Q[:, h, ns], pq[:])

            def proj_v_pair(i):
                pv = plpool.tile([128, 2, 512], F32, tag="pl", name=f"pv_{i}")
                for half in range(2):
                    yt = 2 * i + half
                    ys = slice(yt * 128, (yt + 1) * 128)
                    nc.tensor.matmul(
                        pv[:, half, :], Y[:, 0, ys], WVT[:, 0, :], start=True, stop=False
                    )
                    nc.tensor.matmul(
                        pv[:, half, :], Y[:, 1, ys], WVT[:, 1, :], start=False, stop=True
                    )
                nc.vector.tensor_copy(VT[:, 2 * i : 2 * i + 2, :], pv[:])

            proj_qk_half(0, 0)
            proj_qk_half(0, 1)
            for i in range(NYT // 2):
                proj_v_pair(i)

            # proj_qk(h) halves interleaved at steps 4(h-1)+1 and 4(h-1)+2,
            # well before slot 4h's phase_a is emitted at step 4h-1.
            proj_chunks = {}
            for h in range(1, HPC):
                proj_chunks[4 * (h - 1) + 1] = (h, 0)
                proj_chunks[4 * (h - 1) + 2] = (h, 1)

            # ---- attention, software-pipelined one slot deep ----------------
            # Phase A(s): QK^T + exp -> E mega-tile.  Phase B(s): AV, then
            # denominator (DVE add-tree + one broadcast matmul), normalize.
            # Emitting A(s+1) before B(s) keeps ScalarE fed while the PE
            # drains the previous slot's accumulations.
            slots = [(xblk, h) for h in range(HPC) for xblk in range(NXB)]
            att_tiles = {}

            # Denominator leaf adds (level 1 of the add-tree) fire as soon as
            # the exps they consume have landed: one on DVE, three on GpSimd.
            # Emitting them inline keeps the tree's tail short so the
            # broadcast matmul two steps later never stalls the PE.
            LEAF = {1: 0, 3: 1, 5: 2, 7: 3}

            def _a_range(s, ET, g0, g1):
                xblk, h = slots[s]
                xs = slice(xblk * 512, (xblk + 1) * 512)
                E, T4 = ET
                for g in range(g0, g1):
                    pl = plpool.tile([128, 1024], F32, tag="pl", name=f"pl_{s}_{g}")
                    for j in range(2):
                        yt = 2 * g + j
                        nc.tensor.matmul(
                            pl[:, j * 512 : (j + 1) * 512],
                            K[:, h, yt * 128 : (yt + 1) * 128],
                            Q[:, h, xs],
                            start=True,
                            stop=True,
                        )
                    nc.scalar.activation(
                        E[:, 2 * g : 2 * g + 2, :], pl[:], EXPF, scale=SCALE
                    )
                    if g in LEAF:
                        i = LEAF[g]
                        eng = nc.vector if g == 1 else nc.gpsimd
                        eng.tensor_add(
                            T4[:, i],
                            E[:, 4 * i : 4 * i + 2, :],
                            E[:, 4 * i + 2 : 4 * i + 4, :],
                        )

            def phase_a_head(s):
                """QK+exp for the first 3 of 8 logit tiles. Emitted before the
                previous slot's AV block so ScalarE stays fed while the PE is
                busy with AV; the remaining 5 tiles follow in phase_a_tail."""
                E = epool.tile([128, NYT, 512], F16, tag="E", name=f"E_{s}")
                T4 = t4pool.tile([128, 4, 2, 512], F16, tag="T4", name=f"T4_{s}")
                ET = (E, T4)
                _a_range(s, ET, 0, 3)
                return ET

            def phase_a_tail(s, ET):
                _a_range(s, ET, 3, 8)

            def phase_b1(s, ET):
                """AV accumulation + denominator tree levels 2..4 on DVE."""
                E, T4 = ET
                xblk, h = slots[s]
                hs = slice(h * 128, (h + 1) * 128)
                po = popool.tile([128, 512], F32, tag="po", name=f"pav_{s}")
                for yt in range(NYT):
                    nc.tensor.matmul(
                        po[:],
                        VT[:, yt, hs],
                        E[:, yt, :],
                        start=(yt == 0),
                        stop=(yt == NYT - 1),
                    )
                T2 = t2pool.tile([128, 2, 2, 512], F16, tag="T2", name=f"T2_{s}")
                nc.vector.tensor_add(T2[:, 0], T4[:, 0], T4[:, 1])
                nc.vector.tensor_add(T2[:, 1], T4[:, 2], T4[:, 3])
                T1 = t1pool.tile([128, 2, 512], F16, tag="T1", name=f"T1_{s}")
                nc.vector.tensor_add(T1[:], T2[:, 0], T2[:, 1])
                TS = tspool.tile([128, 512], F16, tag="TS", name=f"TS_{s}")
                nc.vector.tensor_add(TS[:], T1[:, 0, :], T1[:, 1, :])
                po_tiles[s] = po
                ts_tiles[s] = TS

            def phase_b2(s):
                """Broadcast sum_y across partitions (one ones-matmul),
                normalize, and the Wo projection once a head group closes.
                Runs one step after b1(s) so the add-tree is long done."""
                xblk, h = slots[s]
                if h == 0:
                    att_tiles[xblk] = attpool.tile(
                        [128, HPC, 512], F16, tag="ATT", name=f"ATT_{xblk}"
                    )
                ATT = att_tiles[xblk]
                pd = pdpool.tile([128, 512], F32, tag="pd", name=f"pden_{s}")
                nc.tensor.matmul(pd[:], ONES[:], ts_tiles[s][:], start=True, stop=True)
                rc = rcpool.tile([128, 512], F32, tag="rc", name=f"rc_{s}")
                nc.vector.reciprocal(rc[:], pd[:])
                nc.vector.tensor_mul(ATT[:, h, :], po_tiles[s][:], rc[:])
                if h == HPC - 1:
                    xs = slice(xblk * 512, (xblk + 1) * 512)
                    pw = pdpool.tile([128, 512], F32, tag="pd", name=f"pw_{xblk}")
                    for hh in range(HPC):
                        nc.tensor.matmul(
                            pw[:],
                            WOT[:, hh, :],
                            ATT[:, hh, :],
                            start=(hh == 0),
                            stop=(hh == HPC - 1),
                        )
                    ob = smallpool.tile([128, 512], F32, tag="osb", name=f"ob_{xblk}")
                    nc.vector.tensor_copy(ob[:], pw[:])
                    nc.sync.dma_start(out[:, xs], ob[:])

            po_tiles, ts_tiles = {}, {}
            ets = {0: phase_a_head(0)}
            phase_a_tail(0, ets[0])
            for s in range(1, len(slots)):
                ets[s] = phase_a_head(s)
                if s >= 2:
                    phase_b2(s - 2)
                phase_b1(s - 1, ets.pop(s - 1))
                if s in proj_chunks:
                    h, half = proj_chunks[s]
                    proj_qk_half(h, half)
                phase_a_tail(s, ets[s])
            phase_b1(len(slots) - 1, ets.pop(len(slots) - 1))
            phase_b2(len(slots) - 2)
            phase_b2(len(slots) - 1)

    _split_multi_waits(nc)
    return nc


_NC = None


def _get_nc():
    global _NC
    if _NC is None:
        _NC = _build_nc()
    return _NC


def kernel(x, y, Wq, Wk, Wv, Wo):
    global LAST_EXEC_NS, LAST_RESULTS
    x = np.asarray(x, dtype=np.float32)
    y = np.asarray(y, dtype=np.float32)
    Wq3 = np.asarray(Wq, dtype=np.float32).reshape(H, D, C)
    Wk3 = np.asarray(Wk, dtype=np.float32).reshape(H, D, C)
    Wv3 = np.asarray(Wv, dtype=np.float32).reshape(H, D, C)
    Wo2 = np.asarray(Wo, dtype=np.float32)  # [D, H*D]

    in_maps = []
    for core in range(NCORES):
        b, g = core // 2, core % 2
        hsl = slice(4 * g, 4 * g + HPC)
        wqt = Wq3[hsl].reshape(HPC * D, C).T  # [c, hd]
        wkt = Wk3[hsl].reshape(HPC * D, C).T
        wvt = Wv3[hsl].reshape(HPC * D, C).T
        wot = Wo2[:, 4 * g * D : (4 * g + HPC) * D].T  # [hd, o]
        wpack = np.concatenate([wkt, wqt, wvt], axis=1)  # [c, 3*hd]
        in_maps.append(
            {
                "xb": np.ascontiguousarray(x[b]).astype(np.float16),
                "yb": np.ascontiguousarray(y[b]).astype(np.float16),
                "wpack": np.ascontiguousarray(wpack).astype(np.float16),
                "wot": np.ascontiguousarray(wot).astype(np.float16),
            }
        )

    import os

    trace = bool(int(os.environ.get("ATTN_TRACE", "0")))
    res = run_bass_kernel_spmd(
        _get_nc(), in_maps, core_ids=list(range(NCORES)), trace=trace
    )
    LAST_EXEC_NS = res.exec_time_ns
    LAST_RESULTS = res

    out = np.empty((B, D, N), dtype=np.float32)
    for b in range(B):
        out[b] = res.results[2 * b]["out"] + res.results[2 * b + 1]["out"]
    return out
